# revision 7
# baseline (speedup 1.0000x reference)
"""Matrix NMS (SOLOv2 gaussian decay) on 8 TRN2 NeuronCores.

Strategy: shard the pixel (h*w=40960) contraction dim across the 8 cores.
Each core receives X_c = flat.T[c*5120:(c+1)*5120] (5120 x 1024, f32), casts
to bf16 in SBUF (exact for binary masks), and computes the partial Gram
matrix X_c.T @ X_c on the PE.  A ReduceScatter(add) sums the partials and
hands each core its 128-row stripe of the full intersection matrix.  mask
areas ride along as an extra row per shard (diag of the Gram = area, since
the masks are 0/1).  The epilogue exploits Gram symmetry: row j of the
stripe is also column j, so both compensate_iou (column max) and the final
min-reduction become free-dim reductions; one 512B AllGather distributes
compensate_iou.  All core-dependent constants (triu/label mask, diagonal
selector, score slice) are host-prepared inputs, so the SPMD program is
identical on every core.
"""

import os
import sys

import numpy as np

for _p in ("/opt/trn_rl_repo",):
    if _p not in sys.path:
        sys.path.insert(0, _p)

from concourse import bacc, bass, mybir, tile
from concourse import bass_utils

N = 1024           # candidates
HWPIX = 160 * 256  # 40960 pixels
W = 8              # cores
KC = HWPIX // W    # 5120 pixel-slice per core
KT = KC // 128     # 40 k-tiles of 128
RB = N // W        # 128-row output stripe per core
SIGMA = 2.0

F32 = mybir.dt.float32
BF16 = mybir.dt.bfloat16

# Gram production: upper-triangular blocks + PE-transpose mirror (saves ~44%
# of PE work vs computing all 64 blocks).
UPPER_MIRROR = True


def build_nc():
    nc = bacc.Bacc("TRN2", target_bir_lowering=False, debug=False, num_devices=W)

    xT = nc.dram_tensor("xT", [KC, N], F32, kind="ExternalInput")
    maskT_h = nc.dram_tensor("maskT", [RB, N], F32, kind="ExternalInput")
    diagsel_h = nc.dram_tensor("diagsel", [RB, N], F32, kind="ExternalInput")
    scores_h = nc.dram_tensor("scores", [RB, 1], F32, kind="ExternalInput")
    ident_h = nc.dram_tensor("ident", [128, 128], F32, kind="ExternalInput")
    ones_h = nc.dram_tensor("ones_r", [1, 128], F32, kind="ExternalInput")
    out_h = nc.dram_tensor("out", [RB, 1], F32, kind="ExternalOutput")

    RG = [list(range(W))]

    with tile.TileContext(nc) as tc:
        with (
            tc.tile_pool(name="dram", bufs=1, space="DRAM") as dramp,
            tc.tile_pool(name="xp", bufs=1) as xp,
            tc.tile_pool(name="stage", bufs=4) as stp,
            tc.tile_pool(name="pg", bufs=2, space="PSUM") as pgp,
            tc.tile_pool(name="tp", bufs=2, space="PSUM") as tpp,
            tc.tile_pool(name="epi_ps", bufs=1, space="PSUM") as epp,
            tc.tile_pool(name="gb", bufs=3) as gbp,
            tc.tile_pool(name="mir", bufs=3) as mirp,
            tc.tile_pool(name="sc", bufs=1) as scp,
            tc.tile_pool(name="epi", bufs=1) as ep,
        ):
            # internal DRAM buffers for collectives
            cc_in = dramp.tile([W * (RB + 1), N], F32, tag="cc_in")
            cc_out = dramp.tile([RB + 1, N], F32, tag="cc_out")
            ag_in = dramp.tile([RB, 1], F32, tag="ag_in")
            ag_out = dramp.tile([N, 1], F32, tag="ag_out")
            s_scr = dramp.tile([1, N], F32, tag="s_scr")

            # small constants
            ident = scp.tile([128, 128], F32, tag="ident")
            nc.sync.dma_start(ident[:], ident_h[:])
            ones_r = scp.tile([1, 128], F32, tag="ones_r")
            nc.sync.dma_start(ones_r[:], ones_h[:])
            s_all = scp.tile([128, W], F32, tag="s_all")

            # ---- phase 1: stream X in, cast f32 -> bf16, keep resident in SBUF
            xs = []
            for t in range(KT):
                st = stp.tile([128, N], F32, tag="stage")
                nc.sync.dma_start(st[:], xT[t * 128 : (t + 1) * 128, :])
                xb = xp.tile([128, N], BF16, tag=f"x{t}")
                nc.vector.tensor_copy(xb[:], st[:])
                xs.append(xb)

            # ---- phase 2: Gram blocks
            for a in range(W):
                lo = a * 128 if UPPER_MIRROR else 0
                wdt = N - lo
                pg = pgp.tile([128, wdt], F32, tag="pg")
                for t in range(KT):
                    lhsT = xs[t][:, a * 128 : (a + 1) * 128]
                    for off in range(0, wdt, 512):
                        cw = min(512, wdt - off)
                        nc.tensor.matmul(
                            pg[:, off : off + cw],
                            lhsT,
                            xs[t][:, lo + off : lo + off + cw],
                            start=(t == 0),
                            stop=(t == KT - 1),
                        )
                gb = gbp.tile([128, wdt], F32, tag="gb")
                nc.vector.tensor_copy(gb[:], pg[:])
                nc.sync.dma_start(
                    cc_in[a * (RB + 1) : a * (RB + 1) + 128, lo : lo + wdt], gb[:]
                )
                # diagonal 128x128 block -> partial mask areas (X is 0/1 so
                # diag(Gram) = area); s_all[p, a] = s_partial[128a + p]
                dmul = gbp.tile([128, 128], F32, tag="dmul")
                nc.vector.tensor_mul(dmul[:], gb[:, a * 128 - lo : a * 128 - lo + 128], ident[:])
                nc.vector.tensor_reduce(
                    s_all[:, a : a + 1], dmul[:], axis=mybir.AxisListType.X,
                    op=mybir.AluOpType.add,
                )
                if UPPER_MIRROR:
                    # mirror off-diagonal blocks (a,b)->(b,a) via PE transpose
                    for b in range(a + 1, W):
                        tp = tpp.tile([128, 128], F32, tag="tp")
                        nc.tensor.transpose(
                            tp[:], gb[:, b * 128 - lo : b * 128 - lo + 128], ident[:]
                        )
                        mb = mirp.tile([128, 128], F32, tag="mb")
                        nc.vector.tensor_copy(mb[:], tp[:])
                        nc.sync.dma_start(
                            cc_in[b * (RB + 1) : b * (RB + 1) + 128, a * 128 : a * 128 + 128],
                            mb[:],
                        )

            # partial areas -> s_scr (linear, candidate order), then into the
            # per-shard area rows of cc_in
            for a in range(W):
                nc.sync.dma_start(s_scr[:, a * 128 : (a + 1) * 128], s_all[:, a : a + 1])
            for r in range(W):
                nc.sync.dma_start(cc_in[r * (RB + 1) + RB : r * (RB + 1) + RB + 1, :], s_scr[:])

            # ---- ReduceScatter(add): each core gets its 129-row shard
            nc.gpsimd.collective_compute(
                "ReduceScatter",
                mybir.AluOpType.add,
                replica_groups=RG,
                ins=[cc_in[:].opt()],
                outs=[cc_out[:].opt()],
            )

            # ---- epilogue on the stripe
            stripe = ep.tile([128, N], F32, tag="stripe")
            nc.sync.dma_start(stripe[:], cc_out[0:RB, :])
            srow = ep.tile([1, N], F32, tag="srow")
            nc.sync.dma_start(srow[:], cc_out[RB : RB + 1, :])
            maskT = ep.tile([128, N], F32, tag="maskT")
            nc.sync.dma_start(maskT[:], maskT_h[:])
            diagsel = ep.tile([128, N], F32, tag="diagsel")
            nc.sync.dma_start(diagsel[:], diagsel_h[:])
            scores = ep.tile([128, 1], F32, tag="scores")
            nc.sync.dma_start(scores[:], scores_h[:])

            # s_col[p] = area of row (128c+p)  (diagonal of the stripe)
            tmp = ep.tile([128, N], F32, tag="tmp")
            nc.vector.tensor_mul(tmp[:], stripe[:], diagsel[:])
            s_col = ep.tile([128, 1], F32, tag="s_col")
            nc.vector.tensor_reduce(
                s_col[:], tmp[:], axis=mybir.AxisListType.X, op=mybir.AluOpType.add
            )

            # broadcast s (row) across partitions via k=1 outer matmul
            sj = epp.tile([128, N], F32, tag="eps")
            for off in range(0, N, 512):
                nc.tensor.matmul(
                    sj[:, off : off + 512], ones_r[:], srow[:, off : off + 512],
                    start=True, stop=True,
                )
            # union = s_i + s_j - inter
            un = ep.tile([128, N], F32, tag="un")
            nc.vector.tensor_tensor(un[:], sj[:], stripe[:], op=mybir.AluOpType.subtract)
            nc.vector.tensor_scalar_add(un[:], un[:], s_col[:])
            rec = ep.tile([128, N], F32, tag="rec")
            nc.vector.reciprocal(rec[:], un[:])
            # dmT[p, i] = d[i, 128c+p]  (masked IoU, transposed view via symmetry)
            dmT = ep.tile([128, N], F32, tag="dmT")
            nc.vector.tensor_mul(dmT[:], stripe[:], rec[:])
            nc.vector.tensor_mul(dmT[:], dmT[:], maskT[:])
            # compensate_iou for this core's 128 candidates: free-dim max
            c_loc = ep.tile([128, 1], F32, tag="c_loc")
            nc.vector.tensor_reduce(
                c_loc[:], dmT[:], axis=mybir.AxisListType.X, op=mybir.AluOpType.max
            )
            nc.sync.dma_start(ag_in[:], c_loc[:])
            nc.gpsimd.collective_compute(
                "AllGather",
                mybir.AluOpType.bypass,
                replica_groups=RG,
                ins=[ag_in[:].opt()],
                outs=[ag_out[:].opt()],
            )
            crow = ep.tile([1, N], F32, tag="crow")
            nc.sync.dma_start(crow[:], ag_out[:])
            c2row = ep.tile([1, N], F32, tag="c2row")
            nc.vector.tensor_mul(c2row[:], crow[:], crow[:])
            c2b = epp.tile([128, N], F32, tag="eps")
            for off in range(0, N, 512):
                nc.tensor.matmul(
                    c2b[:, off : off + 512], ones_r[:], c2row[:, off : off + 512],
                    start=True, stop=True,
                )
            # f[p, i] = d[i, j]^2 - c[i]^2 ; M_j = max_i f  (j = 128c+p)
            f = ep.tile([128, N], F32, tag="f")
            nc.vector.tensor_mul(f[:], dmT[:], dmT[:])
            nc.vector.tensor_tensor(f[:], f[:], c2b[:], op=mybir.AluOpType.subtract)
            m_loc = ep.tile([128, 1], F32, tag="m_loc")
            nc.vector.tensor_reduce(
                m_loc[:], f[:], axis=mybir.AxisListType.X, op=mybir.AluOpType.max
            )
            # out = scores * exp(-sigma * M)
            e_t = ep.tile([128, 1], F32, tag="e_t")
            nc.scalar.activation(
                e_t[:], m_loc[:], mybir.ActivationFunctionType.Exp, scale=-SIGMA
            )
            outsb = ep.tile([128, 1], F32, tag="outsb")
            nc.vector.tensor_mul(outsb[:], e_t[:], scores[:])
            nc.sync.dma_start(out_h[:], outsb[:])

    nc.compile()
    return nc


_NC_CACHE = None


def _get_nc():
    global _NC_CACHE
    if _NC_CACHE is None:
        _NC_CACHE = build_nc()
    return _NC_CACHE


def make_in_maps(seg_masks, cate_labels, cate_scores):
    flat = np.ascontiguousarray(np.asarray(seg_masks, dtype=np.float32).reshape(N, -1))
    labels = np.asarray(cate_labels)
    scores = np.asarray(cate_scores, dtype=np.float32)
    xTfull = np.ascontiguousarray(flat.T)  # (40960, 1024)
    gidx = np.arange(N)
    ident = np.eye(128, dtype=np.float32)
    ones_r = np.ones((1, 128), dtype=np.float32)
    in_maps = []
    for c in range(W):
        rows = slice(c * RB, (c + 1) * RB)
        gr = gidx[rows]
        maskT = (
            (gidx[None, :] < gr[:, None]) & (labels[None, :] == labels[rows][:, None])
        ).astype(np.float32)
        diagsel = np.zeros((RB, N), dtype=np.float32)
        diagsel[np.arange(RB), gr] = 1.0
        in_maps.append(
            {
                "xT": xTfull[c * KC : (c + 1) * KC],
                "maskT": maskT,
                "diagsel": diagsel,
                "scores": scores[rows].reshape(RB, 1),
                "ident": ident,
                "ones_r": ones_r,
            }
        )
    return in_maps


def run_device(in_maps, trace=False):
    nc = _get_nc()
    res = bass_utils.run_bass_kernel_spmd(
        nc, in_maps, core_ids=list(range(W)), trace=trace
    )
    return res


def make_runner(in_maps, nc=None):
    """Device-resident repeated executor (mirrors bass2jax.run_bass_via_pjrt
    multi-core path, but keeps the big inputs on device across calls)."""
    import jax
    from jax.sharding import Mesh, NamedSharding, PartitionSpec
    from jax.experimental.shard_map import shard_map
    from concourse import bass2jax

    if nc is None:
        nc = _get_nc()
    bass2jax.install_neuronx_cc_hook()

    partition_name = nc.partition_id_tensor.name if nc.partition_id_tensor else None
    in_names, out_names, out_avals, zero_outs = [], [], [], []
    for alloc in nc.m.functions[0].allocations:
        if not isinstance(alloc, mybir.MemoryLocationSet):
            continue
        name = alloc.memorylocations[0].name
        if alloc.kind == "ExternalInput":
            if name != partition_name:
                in_names.append(name)
        elif alloc.kind == "ExternalOutput":
            out_names.append(name)
            shape = tuple(alloc.tensor_shape)
            dtype = mybir.dt.np(alloc.dtype)
            out_avals.append(jax.core.ShapedArray(shape, dtype))
            zero_outs.append(np.zeros(shape, dtype))
    n_params = len(in_names)
    n_outs = len(out_avals)
    all_in_names = list(in_names) + list(out_names)
    if partition_name is not None:
        all_in_names.append(partition_name)
    donate = tuple(range(n_params, n_params + n_outs))

    def _body(*args):
        operands = list(args)
        if partition_name is not None:
            operands.append(bass2jax.partition_id_tensor())
        outs = bass2jax._bass_exec_p.bind(
            *operands,
            out_avals=tuple(out_avals),
            in_names=tuple(all_in_names),
            out_names=tuple(out_names),
            lowering_input_output_aliases=(),
            sim_require_finite=True,
            sim_require_nnan=True,
            nc=nc,
        )
        return tuple(outs)

    ncore = len(in_maps)
    devices = jax.devices()[:ncore]
    mesh = Mesh(np.asarray(devices), ("core",))
    in_specs = (PartitionSpec("core"),) * (n_params + n_outs)
    out_specs = (PartitionSpec("core"),) * n_outs
    sharded = jax.jit(
        shard_map(_body, mesh=mesh, in_specs=in_specs, out_specs=out_specs,
                  check_rep=False),
        donate_argnums=donate, keep_unused=True,
    )
    shd = NamedSharding(mesh, PartitionSpec("core"))
    concat_in = [
        np.concatenate([np.asarray(in_maps[c][nm]) for c in range(ncore)], axis=0)
        for nm in in_names
    ]
    dev_in = [jax.device_put(x, shd) for x in concat_in]
    zshapes = [(ncore * z.shape[0], *z.shape[1:]) for z in zero_outs]

    def run():
        dz = [jax.device_put(np.zeros(s, z.dtype), shd)
              for s, z in zip(zshapes, zero_outs)]
        return sharded(*dev_in, *dz)

    def fetch(out_arrs):
        return [
            {nm: np.asarray(out_arrs[i]).reshape(ncore, *out_avals[i].shape)[c]
             for i, nm in enumerate(out_names)}
            for c in range(ncore)
        ]

    return run, fetch


def build_null_nc():
    """Trivial 8-core kernel (one 4KB DMA through SBUF) to measure the
    dispatch + launch floor of the execute path."""
    nc = bacc.Bacc("TRN2", target_bir_lowering=False, debug=False, num_devices=W)
    a_h = nc.dram_tensor("a", [RB, 1], F32, kind="ExternalInput")
    o_h = nc.dram_tensor("out", [RB, 1], F32, kind="ExternalOutput")
    with tile.TileContext(nc) as tc:
        with tc.tile_pool(name="p", bufs=1) as p:
            t = p.tile([RB, 1], F32, tag="t")
            nc.sync.dma_start(t[:], a_h[:])
            nc.sync.dma_start(o_h[:], t[:])
    nc.compile()
    return nc


def kernel(seg_masks, cate_labels, cate_scores):
    in_maps = make_in_maps(seg_masks, cate_labels, cate_scores)
    res = run_device(in_maps)
    outs = [np.asarray(res.results[c]["out"]).reshape(RB) for c in range(W)]
    return np.concatenate(outs).astype(np.float32)


# revision 33
# speedup vs baseline: 11.1442x; 11.1442x over previous
"""Matrix NMS (SOLOv2 gaussian decay) on 8 TRN2 NeuronCores.

Strategy: shard the pixel (h*w=40960) contraction dim across the 8 cores.
Each core receives X_c = flat.T[c*5120:(c+1)*5120] (5120 x 1024, f32),
DMA-casts it to bf16 in SBUF (exact for binary masks), and computes the
partial Gram matrix X_c.T @ X_c on the PE (upper blocks + PE-transpose
mirror).  Partials ship as int16 (exact: per-core intersections <= 5120)
through an AllToAll; each core then tree-sums the 8 partials of its own
128-row stripe on the vector engine (AllToAll + local add beats
ReduceScatter here: the CCE reduce path runs at half the wire rate and
f32 doubles the bytes).  Mask areas ride along as an extra row per shard
(diag of the Gram = area for 0/1 masks).  The epilogue exploits Gram
symmetry: row j of the stripe is also column j, so compensate_iou (column
max) and the final min-reduction both become free-dim reductions; one 512B
AllGather distributes compensate_iou.  All core-dependent constants
(triu/label mask, diagonal selector, score slice) are host-prepared
inputs, so the SPMD program is identical on every core.
"""

import sys

import numpy as np

for _p in ("/opt/trn_rl_repo",):
    if _p not in sys.path:
        sys.path.insert(0, _p)

from concourse import bacc, bass, mybir, tile
from concourse import bass_utils

N = 1024           # candidates
HWPIX = 160 * 256  # 40960 pixels
W = 8              # cores
KC = HWPIX // W    # 5120 pixel-slice per core
KT = KC // 128     # 40 k-tiles of 128
GRP = 4            # k-tiles per resident SBUF group
RB = N // W        # 128-row output stripe per core
SR = RB + 1        # shard rows: 128 gram rows + 1 area row
SIGMA = 2.0

F32 = mybir.dt.float32
FP8 = mybir.dt.float8e4  # e4m3: exact for 0/1 mask values
I16 = mybir.dt.int16


def build_nc(variant="full"):
    # variant: "full" = real kernel; "nocc" = collectives replaced by local
    # DMA copies (wrong math, identical local compute/DMA — timing/sim only)
    nc = bacc.Bacc(
        "TRN2", target_bir_lowering=False, debug=False,
        num_devices=W if variant == "full" else 1,
    )

    xT = nc.dram_tensor("xT", [128, KT * N], F32, kind="ExternalInput")
    maskT_h = nc.dram_tensor("maskT", [RB, N], F32, kind="ExternalInput")
    diagsel_h = nc.dram_tensor("diagsel", [RB, N], F32, kind="ExternalInput")
    scores_h = nc.dram_tensor("scores", [1, RB], F32, kind="ExternalInput")
    ident_h = nc.dram_tensor("ident", [128, 128], F32, kind="ExternalInput")
    ones_h = nc.dram_tensor("ones_r", [1, 128], F32, kind="ExternalInput")
    out_h = nc.dram_tensor("out", [1, RB], F32, kind="ExternalOutput")

    RG = [list(range(W))]

    with tile.TileContext(nc) as tc:
        with (
            tc.tile_pool(name="dram", bufs=1, space="DRAM") as dramp,
            tc.tile_pool(name="xp", bufs=1) as xp,
            tc.tile_pool(name="stage", bufs=4) as stp,
            tc.tile_pool(name="pg", bufs=4, space="PSUM") as pgp,
            tc.tile_pool(name="gb", bufs=3) as gbp,
            tc.tile_pool(name="a2al", bufs=1) as alp,
            tc.tile_pool(name="sc", bufs=1) as scp,
            tc.tile_pool(name="epi", bufs=1) as ep,
        ):
            cc_in = dramp.tile([W * SR, N], I16, tag="cc_in")
            a2a_out = dramp.tile([W * SR, N], I16, tag="a2a_out")
            ag_in = dramp.tile([RB, 1], F32, tag="ag_in")
            ag_out = dramp.tile([N, 1], F32, tag="ag_out")

            # small constants
            ident = scp.tile([128, 128], F32, tag="ident")
            nc.gpsimd.dma_start(ident[:], ident_h[:])
            ones_r = scp.tile([1, 128], F32, tag="ones_r")
            nc.gpsimd.dma_start(ones_r[:], ones_h[:])
            s_all = scp.tile([128, W], F32, tag="s_all")

            # ---- phase 1: per-k-tile f32 loads (partition-split across DMA
            # queues so early tiles complete early and the PE can chase),
            # DVE cast to bf16, SBUF resident
            xg = [xp.tile([128, GRP, N], FP8, tag=f"x{g}", name=f"xg{g}")
                  for g in range(KT // GRP)]
            LW = 5  # k-tiles per load DMA: 20KB contiguous per descriptor
            for i in range(KT // LW):
                t0 = i * LW
                st = stp.tile([128, LW, N], F32, tag="stage")
                nc.sync.dma_start(st[:], xT[:, t0 * N : (t0 + LW) * N])
                for j in range(LW):
                    t = t0 + j
                    nc.vector.tensor_copy(xg[t // GRP][:, t % GRP, :], st[:, j, :])

            def xs(t):
                return xg[t // GRP][:, t % GRP, :]

            def drain(a, pg):
                """PSUM block row a -> int16 cells of cc_in.

                Cell (a,b) goes to shard b at column block a: the receiver
                transposes its whole summed column-block (the diagonal block
                is symmetric, so a uniform transpose is correct SPMD-wide).
                """
                lo = a * 128
                wdt = N - lo
                gb16 = gbp.tile([128, wdt], I16, tag="gb16")
                nc.vector.tensor_copy(gb16[:], pg[:, :wdt])
                dst = cc_in[a * SR : W * SR, lo : lo + 128].rearrange(
                    "(b r) q -> r b q", r=SR
                )[0:128, :, :]
                nc.sync.dma_start(dst, gb16[:])
                # diag block -> partial areas (X is 0/1 so diag(Gram) = area)
                dmul = gbp.tile([128, 128], F32, tag="dmul")
                nc.vector.tensor_mul(dmul[:], pg[:, 0:128], ident[:])
                nc.vector.tensor_reduce(
                    s_all[:, a : a + 1], dmul[:], axis=mybir.AxisListType.X,
                    op=mybir.AluOpType.add,
                )

            # fp8 DoubleRow: one matmul consumes a PAIR of adjacent k-tiles
            # ([K,2,*] APs), streaming 2 rows/cycle
            NP = KT // 2

            def xpair(q, c0, c1):
                t = 2 * q
                g, j = t // GRP, t % GRP
                return xg[g][:, j : j + 2, c0:c1]

            def gram_pair(pg, a, q):
                wdt = N - a * 128
                lhsT = xpair(q, a * 128, (a + 1) * 128)
                for off in range(0, wdt, 512):
                    cw = min(512, wdt - off)
                    nc.tensor.matmul(
                        pg[:, off : off + cw],
                        lhsT,
                        xpair(q, a * 128 + off, a * 128 + off + cw),
                        start=(q == 0),
                        stop=(q == NP - 1),
                        perf_mode=mybir.MatmulPerfMode.DoubleRow,
                    )

            # ---- phase 2: Gram upper blocks in two PSUM waves
            # wave A (a=0..3, 8 banks) accumulates per arriving k-tile pair so
            # the PE chases the load DMAs; wave B (a=4..7) runs after.
            wave_a = [pgp.tile([128, N - a * 128], F32, tag="pg", name=f"pgA{a}") for a in range(4)]
            for q in range(NP):
                for a in range(4):
                    gram_pair(wave_a[a], a, q)
            for a in range(4):
                drain(a, wave_a[a])
            for a in range(4, W):
                pg = pgp.tile([128, N - a * 128], F32, tag="pg")
                for q in range(NP):
                    gram_pair(pg, a, q)
                drain(a, pg)

            # partial areas: transpose (128,W) -> (W,128) on the PE so the
            # area rows leave SBUF as contiguous 256B rows, not 4B gathers
            s_ps = pgp.tile([W, 128], F32, tag="pg", name="s_ps")
            nc.tensor.transpose(s_ps[:], s_all[:], ident[:])
            s_rt = scp.tile([W, 128], I16, tag="s_rt")
            nc.vector.tensor_copy(s_rt[:], s_ps[:])
            for r in range(W):
                nc.scalar.dma_start(cc_in[r * SR + RB : r * SR + RB + 1, :], s_rt[:])

            # ---- AllToAll: shard s of a2a_out = core s's partial of MY shard
            if variant == "full":
                nc.gpsimd.collective_compute(
                    "AllToAll",
                    mybir.AluOpType.bypass,
                    replica_groups=RG,
                    ins=[cc_in[:].opt()],
                    outs=[a2a_out[:].opt()],
                )
            else:
                for s in range(W):
                    nc.sync.dma_start(
                        a2a_out[s * SR : (s + 1) * SR, :], cc_in[s * SR : (s + 1) * SR, :]
                    )

            # ---- local tree-sum of the 8 partials of this core's column-block
            # (two 3D-AP DMAs pull shards 0-3 / 4-7 side by side in the free
            # dim; 3 halving adds reduce them)
            srow = ep.tile([1, N], F32, tag="srow")
            la = alp.tile([RB, 4, N], I16, tag="ld16")
            nc.scalar.dma_start(
                la[:], a2a_out[0 : 4 * SR, :].rearrange("(s p) n -> p s n", p=SR)[0:RB, :, :]
            )
            lb = alp.tile([RB, 4, N], I16, tag="ld16b")
            nc.sync.dma_start(
                lb[:], a2a_out[4 * SR : 8 * SR, :].rearrange("(s p) n -> p s n", p=SR)[0:RB, :, :]
            )
            p4 = ep.tile([RB, 4, N], I16, tag="p4")
            nc.vector.tensor_add(p4[:], la[:], lb[:])
            p2 = ep.tile([RB, 2, N], I16, tag="p2")
            nc.vector.tensor_add(p2[:], p4[:, 0:2, :], p4[:, 2:4, :])
            summ = ep.tile([128, N], F32, tag="summ")
            nc.vector.tensor_add(summ[:], p2[:, 0, :], p2[:, 1, :])
            # area rows: sum the 8 partials with a k=8 matmul (partition
            # contraction), avoiding partition-offset slices
            ar16 = ep.tile([W, N], I16, tag="ar16")
            for s in range(W):
                nc.scalar.dma_start(ar16[s : s + 1, :], a2a_out[s * SR + RB : (s + 1) * SR, :])
            arf = ep.tile([W, N], F32, tag="arf")
            nc.vector.tensor_copy(arf[:], ar16[:])
            ones8 = ep.tile([W, 1], F32, tag="ones8")
            nc.vector.memset(ones8[:], 1.0)
            arp = pgp.tile([1, N], F32, tag="pg", name="arp")
            for off in range(0, N, 512):
                nc.tensor.matmul(
                    arp[:, off : off + 512], ones8[:], arf[:, off : off + 512],
                    start=True, stop=True,
                )
            nc.vector.tensor_copy(srow[:], arp[:])

            # transpose the summed column-block into this core's row stripe
            # (4 blocks per PSUM tile -> 2 big copies instead of 8 small)
            stripe = ep.tile([128, N], F32, tag="stripe")
            for h in range(2):
                tp = pgp.tile([128, 512], F32, tag="pg", name=f"tph{h}")
                for b in range(4):
                    nc.tensor.transpose(
                        tp[:, b * 128 : (b + 1) * 128],
                        summ[:, (4 * h + b) * 128 : (4 * h + b + 1) * 128],
                        ident[:],
                    )
                nc.vector.tensor_copy(stripe[:, h * 512 : (h + 1) * 512], tp[:])

            # ---- epilogue on the stripe
            maskT = ep.tile([128, N], F32, tag="maskT")
            nc.gpsimd.dma_start(maskT[:], maskT_h[:])
            diagsel = ep.tile([128, N], F32, tag="diagsel")
            nc.gpsimd.dma_start(diagsel[:], diagsel_h[:])
            scores = ep.tile([1, RB], F32, tag="scores")
            nc.gpsimd.dma_start(scores[:], scores_h[:])

            # s_col[p] = area of row (128c+p)  (diagonal of the stripe)
            tmp = ep.tile([128, N], F32, tag="e1")
            nc.vector.tensor_mul(tmp[:], stripe[:], diagsel[:])
            s_col = ep.tile([128, 1], F32, tag="s_col")
            nc.vector.tensor_reduce(
                s_col[:], tmp[:], axis=mybir.AxisListType.X, op=mybir.AluOpType.add
            )

            # broadcast s (row) across partitions via k=1 outer matmul
            sj = pgp.tile([128, N], F32, tag="pg")
            for off in range(0, N, 512):
                nc.tensor.matmul(
                    sj[:, off : off + 512], ones_r[:], srow[:, off : off + 512],
                    start=True, stop=True,
                )
            # union = s_i + s_j - inter
            un = ep.tile([128, N], F32, tag="e2")
            nc.vector.tensor_tensor(un[:], sj[:], stripe[:], op=mybir.AluOpType.subtract)
            nc.vector.tensor_scalar(
                un[:], un[:], s_col[:], 1.0,
                op0=mybir.AluOpType.add, op1=mybir.AluOpType.max,
            )
            rec = ep.tile([128, N], F32, tag="e1")
            nc.vector.reciprocal(rec[:], un[:])
            # dmT[p, i] = d[i, 128c+p]  (masked IoU, transposed view via symmetry)
            dmT = ep.tile([128, N], F32, tag="e2")
            nc.vector.tensor_mul(dmT[:], stripe[:], rec[:])
            nc.vector.tensor_mul(dmT[:], dmT[:], maskT[:])
            # compensate_iou for this core's 128 candidates: free-dim max
            c_loc = ep.tile([128, 1], F32, tag="c_loc")
            nc.vector.tensor_reduce(
                c_loc[:], dmT[:], axis=mybir.AxisListType.X, op=mybir.AluOpType.max
            )
            cl_ps = pgp.tile([1, 128], F32, tag="pg", name="cl_ps")
            nc.tensor.transpose(cl_ps[:], c_loc[:], ident[:])
            cl_row = ep.tile([1, 128], F32, tag="cl_row")
            nc.vector.tensor_copy(cl_row[:], cl_ps[:])
            nc.scalar.dma_start(ag_in[:], cl_row[:])
            if variant == "full":
                nc.gpsimd.collective_compute(
                    "AllGather",
                    mybir.AluOpType.bypass,
                    replica_groups=RG,
                    ins=[ag_in[:].opt()],
                    outs=[ag_out[:].opt()],
                )
            else:
                for r in range(W):
                    nc.sync.dma_start(ag_out[r * RB : (r + 1) * RB, :], ag_in[:])
            crow = ep.tile([1, N], F32, tag="crow")
            nc.sync.dma_start(crow[:], ag_out[:])
            c2row = ep.tile([1, N], F32, tag="c2row")
            nc.scalar.square(c2row[:], crow[:])
            c2b = pgp.tile([128, N], F32, tag="pg")
            for off in range(0, N, 512):
                nc.tensor.matmul(
                    c2b[:, off : off + 512], ones_r[:], c2row[:, off : off + 512],
                    start=True, stop=True,
                )
            # f[p, i] = d[i, j]^2 - c[i]^2 ; M_j = max_i f  (j = 128c+p)
            f = ep.tile([128, N], F32, tag="e1")
            nc.scalar.square(f[:], dmT[:])
            nc.vector.tensor_tensor(f[:], f[:], c2b[:], op=mybir.AluOpType.subtract)
            m_loc = ep.tile([128, 1], F32, tag="m_loc")
            nc.vector.tensor_reduce(
                m_loc[:], f[:], axis=mybir.AxisListType.X, op=mybir.AluOpType.max
            )
            # out = scores * exp(-sigma * M), assembled in row space so the
            # store is one contiguous 512B descriptor
            ml_ps = pgp.tile([1, 128], F32, tag="pg", name="ml_ps")
            nc.tensor.transpose(ml_ps[:], m_loc[:], ident[:])
            e_t = ep.tile([1, RB], F32, tag="e_t")
            nc.scalar.activation(
                e_t[:], ml_ps[:], mybir.ActivationFunctionType.Exp, scale=-SIGMA
            )
            outsb = ep.tile([1, RB], F32, tag="outsb")
            nc.vector.tensor_mul(outsb[:], e_t[:], scores[:])
            nc.scalar.dma_start(out_h[:], outsb[:])

    nc.compile()
    return nc


_NC_CACHE = {}


def _get_nc(variant="full"):
    if variant not in _NC_CACHE:
        _NC_CACHE[variant] = build_nc(variant)
    return _NC_CACHE[variant]


def make_in_maps(seg_masks, cate_labels, cate_scores):
    flat = np.ascontiguousarray(np.asarray(seg_masks, dtype=np.float32).reshape(N, -1))
    labels = np.asarray(cate_labels)
    scores = np.asarray(cate_scores, dtype=np.float32)
    xTfull = np.ascontiguousarray(flat.T)  # (40960, 1024)
    gidx = np.arange(N)
    ident = np.eye(128, dtype=np.float32)
    ones_r = np.ones((1, 128), dtype=np.float32)
    in_maps = []
    for c in range(W):
        rows = slice(c * RB, (c + 1) * RB)
        gr = gidx[rows]
        maskT = (
            (gidx[None, :] < gr[:, None]) & (labels[None, :] == labels[rows][:, None])
        ).astype(np.float32)
        diagsel = np.zeros((RB, N), dtype=np.float32)
        diagsel[np.arange(RB), gr] = 1.0
        in_maps.append(
            {
                # partition-major: row p holds k-rows {p, 128+p, ...} of this
                # core's slice, so each DMA descriptor moves 20KB contiguous
                "xT": np.ascontiguousarray(
                    xTfull[c * KC : (c + 1) * KC]
                    .reshape(KT, 128, N)
                    .transpose(1, 0, 2)
                ).reshape(128, KT * N),
                "maskT": maskT,
                "diagsel": diagsel,
                "scores": scores[rows].reshape(1, RB),
                "ident": ident,
                "ones_r": ones_r,
            }
        )
    return in_maps


def run_device(in_maps, trace=False):
    nc = _get_nc()
    res = bass_utils.run_bass_kernel_spmd(
        nc, in_maps, core_ids=list(range(W)), trace=trace
    )
    return res


def kernel(seg_masks, cate_labels, cate_scores):
    in_maps = make_in_maps(seg_masks, cate_labels, cate_scores)
    res = run_device(in_maps)
    outs = [np.asarray(res.results[c]["out"]).reshape(RB) for c in range(W)]
    return np.concatenate(outs).astype(np.float32)


def make_runner(in_maps, nc=None):
    """Device-resident repeated executor (mirrors bass2jax.run_bass_via_pjrt
    multi-core path, but keeps the big inputs on device across calls)."""
    import jax
    from jax.sharding import Mesh, NamedSharding, PartitionSpec
    from jax.experimental.shard_map import shard_map
    from concourse import bass2jax

    if nc is None:
        nc = _get_nc()
    bass2jax.install_neuronx_cc_hook()

    partition_name = nc.partition_id_tensor.name if nc.partition_id_tensor else None
    in_names, out_names, out_avals, zero_outs = [], [], [], []
    for alloc in nc.m.functions[0].allocations:
        if not isinstance(alloc, mybir.MemoryLocationSet):
            continue
        name = alloc.memorylocations[0].name
        if alloc.kind == "ExternalInput":
            if name != partition_name:
                in_names.append(name)
        elif alloc.kind == "ExternalOutput":
            out_names.append(name)
            shape = tuple(alloc.tensor_shape)
            dtype = mybir.dt.np(alloc.dtype)
            out_avals.append(jax.core.ShapedArray(shape, dtype))
            zero_outs.append(np.zeros(shape, dtype))
    n_params = len(in_names)
    n_outs = len(out_avals)
    all_in_names = list(in_names) + list(out_names)
    if partition_name is not None:
        all_in_names.append(partition_name)
    donate = tuple(range(n_params, n_params + n_outs))

    def _body(*args):
        operands = list(args)
        if partition_name is not None:
            operands.append(bass2jax.partition_id_tensor())
        outs = bass2jax._bass_exec_p.bind(
            *operands,
            out_avals=tuple(out_avals),
            in_names=tuple(all_in_names),
            out_names=tuple(out_names),
            lowering_input_output_aliases=(),
            sim_require_finite=True,
            sim_require_nnan=True,
            nc=nc,
        )
        return tuple(outs)

    ncore = len(in_maps)
    devices = jax.devices()[:ncore]
    mesh = Mesh(np.asarray(devices), ("core",))
    in_specs = (PartitionSpec("core"),) * (n_params + n_outs)
    out_specs = (PartitionSpec("core"),) * n_outs
    sharded = jax.jit(
        shard_map(_body, mesh=mesh, in_specs=in_specs, out_specs=out_specs,
                  check_rep=False),
        donate_argnums=donate, keep_unused=True,
    )
    shd = NamedSharding(mesh, PartitionSpec("core"))
    concat_in = [
        np.concatenate([np.asarray(in_maps[c][nm]) for c in range(ncore)], axis=0)
        for nm in in_names
    ]
    dev_in = [jax.device_put(x, shd) for x in concat_in]
    zshapes = [(ncore * z.shape[0], *z.shape[1:]) for z in zero_outs]

    def run():
        dz = [jax.device_put(np.zeros(s, z.dtype), shd)
              for s, z in zip(zshapes, zero_outs)]
        return sharded(*dev_in, *dz)

    def fetch(out_arrs):
        return [
            {nm: np.asarray(out_arrs[i]).reshape(ncore, *out_avals[i].shape)[c]
             for i, nm in enumerate(out_names)}
            for c in range(ncore)
        ]

    return run, fetch


def build_null_nc():
    """Trivial 8-core kernel (one 4KB DMA through SBUF) to measure the
    dispatch + launch floor of the execute path."""
    nc = bacc.Bacc("TRN2", target_bir_lowering=False, debug=False, num_devices=W)
    a_h = nc.dram_tensor("a", [RB, 1], F32, kind="ExternalInput")
    o_h = nc.dram_tensor("out", [RB, 1], F32, kind="ExternalOutput")
    with tile.TileContext(nc) as tc:
        with tc.tile_pool(name="p", bufs=1) as p:
            t = p.tile([RB, 1], F32, tag="t")
            nc.sync.dma_start(t[:], a_h[:])
            nc.sync.dma_start(o_h[:], t[:])
    nc.compile()
    return nc


# revision 35
# speedup vs baseline: 11.2893x; 1.0130x over previous
"""Matrix NMS (SOLOv2 gaussian decay) on 8 TRN2 NeuronCores.

Strategy: shard the pixel (h*w=40960) contraction dim across the 8 cores.
Each core receives X_c = flat.T[c*5120:(c+1)*5120] (5120 x 1024, f32),
DMA-casts it to bf16 in SBUF (exact for binary masks), and computes the
partial Gram matrix X_c.T @ X_c on the PE (upper blocks + PE-transpose
mirror).  Partials ship as int16 (exact: per-core intersections <= 5120)
through an AllToAll; each core then tree-sums the 8 partials of its own
128-row stripe on the vector engine (AllToAll + local add beats
ReduceScatter here: the CCE reduce path runs at half the wire rate and
f32 doubles the bytes).  Mask areas ride along as an extra row per shard
(diag of the Gram = area for 0/1 masks).  The epilogue exploits Gram
symmetry: row j of the stripe is also column j, so compensate_iou (column
max) and the final min-reduction both become free-dim reductions; one 512B
AllGather distributes compensate_iou.  All core-dependent constants
(triu/label mask, diagonal selector, score slice) are host-prepared
inputs, so the SPMD program is identical on every core.
"""

import sys

import numpy as np

for _p in ("/opt/trn_rl_repo",):
    if _p not in sys.path:
        sys.path.insert(0, _p)

from concourse import bacc, bass, mybir, tile
from concourse import bass_utils

N = 1024           # candidates
HWPIX = 160 * 256  # 40960 pixels
W = 8              # cores
KC = HWPIX // W    # 5120 pixel-slice per core
KT = KC // 128     # 40 k-tiles of 128
GRP = 4            # k-tiles per resident SBUF group
RB = N // W        # 128-row output stripe per core
SR = RB + 1        # shard rows: 128 gram rows + 1 area row
SIGMA = 2.0

F32 = mybir.dt.float32
FP8 = mybir.dt.float8e4  # e4m3: exact for 0/1 mask values
I16 = mybir.dt.int16


def build_nc(variant="full"):
    # variant: "full" = real kernel; "nocc" = collectives replaced by local
    # DMA copies (wrong math, identical local compute/DMA — timing/sim only)
    nc = bacc.Bacc(
        "TRN2", target_bir_lowering=False, debug=False,
        num_devices=W if variant == "full" else 1,
    )

    xT = nc.dram_tensor("xT", [128, KT * N], F32, kind="ExternalInput")
    maskT_h = nc.dram_tensor("maskT", [RB, N], F32, kind="ExternalInput")
    diagsel_h = nc.dram_tensor("diagsel", [RB, N], F32, kind="ExternalInput")
    scores_h = nc.dram_tensor("scores", [1, RB], F32, kind="ExternalInput")
    ident_h = nc.dram_tensor("ident", [128, 128], F32, kind="ExternalInput")
    ones_h = nc.dram_tensor("ones_r", [1, 128], F32, kind="ExternalInput")
    out_h = nc.dram_tensor("out", [1, RB], F32, kind="ExternalOutput")

    RG = [list(range(W))]

    with tile.TileContext(nc) as tc:
        with (
            tc.tile_pool(name="dram", bufs=1, space="DRAM") as dramp,
            tc.tile_pool(name="xp", bufs=1) as xp,
            tc.tile_pool(name="stage", bufs=4) as stp,
            tc.tile_pool(name="pg", bufs=4, space="PSUM") as pgp,
            tc.tile_pool(name="gb", bufs=3) as gbp,
            tc.tile_pool(name="a2al", bufs=1) as alp,
            tc.tile_pool(name="sc", bufs=1) as scp,
            tc.tile_pool(name="epi", bufs=1) as ep,
        ):
            cc_in = dramp.tile([W * SR, N], I16, tag="cc_in")
            a2a_out = dramp.tile([W * SR, N], I16, tag="a2a_out")
            ag_in = dramp.tile([RB, 1], F32, tag="ag_in")
            ag_out = dramp.tile([N, 1], F32, tag="ag_out")

            # small constants
            ident = scp.tile([128, 128], F32, tag="ident")
            nc.gpsimd.dma_start(ident[:], ident_h[:])
            ones_r = scp.tile([1, 128], F32, tag="ones_r")
            nc.gpsimd.dma_start(ones_r[:], ones_h[:])
            s_all = scp.tile([128, W], F32, tag="s_all")

            # ---- phase 1: per-k-tile f32 loads (partition-split across DMA
            # queues so early tiles complete early and the PE can chase),
            # DVE cast to bf16, SBUF resident
            xg = [xp.tile([128, GRP, N], FP8, tag=f"x{g}", name=f"xg{g}")
                  for g in range(KT // GRP)]
            LW = 5  # k-tiles per load DMA: 20KB contiguous per descriptor
            for i in range(KT // LW):
                t0 = i * LW
                st = stp.tile([128, LW, N], F32, tag="stage")
                nc.sync.dma_start(st[:], xT[:, t0 * N : (t0 + LW) * N])
                for j in range(LW):
                    t = t0 + j
                    nc.vector.tensor_copy(xg[t // GRP][:, t % GRP, :], st[:, j, :])

            def xs(t):
                return xg[t // GRP][:, t % GRP, :]

            def drain(a, pg):
                """PSUM block row a -> int16 cells of cc_in.

                Cell (a,b) goes to shard b at column block a: the receiver
                transposes its whole summed column-block (the diagonal block
                is symmetric, so a uniform transpose is correct SPMD-wide).
                """
                lo = a * 128
                wdt = N - lo
                gb16 = gbp.tile([128, wdt], I16, tag="gb16")
                nc.vector.tensor_copy(gb16[:], pg[:, :wdt])
                dst = cc_in[a * SR : W * SR, lo : lo + 128].rearrange(
                    "(b r) q -> r b q", r=SR
                )[0:128, :, :]
                nc.sync.dma_start(dst, gb16[:])
                # diag block -> partial areas (X is 0/1 so diag(Gram) = area)
                dmul = gbp.tile([128, 128], F32, tag="dmul")
                nc.vector.tensor_mul(dmul[:], pg[:, 0:128], ident[:])
                nc.vector.tensor_reduce(
                    s_all[:, a : a + 1], dmul[:], axis=mybir.AxisListType.X,
                    op=mybir.AluOpType.add,
                )

            # fp8 DoubleRow: one matmul consumes a PAIR of adjacent k-tiles
            # ([K,2,*] APs), streaming 2 rows/cycle
            NP = KT // 2

            def xpair(q, c0, c1):
                t = 2 * q
                g, j = t // GRP, t % GRP
                return xg[g][:, j : j + 2, c0:c1]

            def gram_pair(pg, a, q):
                wdt = N - a * 128
                lhsT = xpair(q, a * 128, (a + 1) * 128)
                for off in range(0, wdt, 512):
                    cw = min(512, wdt - off)
                    nc.tensor.matmul(
                        pg[:, off : off + cw],
                        lhsT,
                        xpair(q, a * 128 + off, a * 128 + off + cw),
                        start=(q == 0),
                        stop=(q == NP - 1),
                        perf_mode=mybir.MatmulPerfMode.DoubleRow,
                    )

            # ---- phase 2: Gram upper blocks in two PSUM waves
            # wave A (a=0..3, 8 banks) accumulates per arriving k-tile pair so
            # the PE chases the load DMAs; wave B (a=4..7) runs after.
            wave_a = [pgp.tile([128, N - a * 128], F32, tag="pg", name=f"pgA{a}") for a in range(4)]
            for q in range(NP):
                for a in range(4):
                    gram_pair(wave_a[a], a, q)
            for a in range(4):
                drain(a, wave_a[a])
            for a in range(4, W):
                pg = pgp.tile([128, N - a * 128], F32, tag="pg")
                for q in range(NP):
                    gram_pair(pg, a, q)
                drain(a, pg)

            # partial areas: transpose (128,W) -> (W,128) on the PE so the
            # area rows leave SBUF as contiguous 256B rows, not 4B gathers
            s_ps = pgp.tile([W, 128], F32, tag="pg", name="s_ps")
            nc.tensor.transpose(s_ps[:], s_all[:], ident[:])
            s_rt = scp.tile([W, 128], I16, tag="s_rt")
            nc.vector.tensor_copy(s_rt[:], s_ps[:])
            for r in range(W):
                nc.scalar.dma_start(cc_in[r * SR + RB : r * SR + RB + 1, :], s_rt[:])

            # ---- AllToAll: shard s of a2a_out = core s's partial of MY shard
            if variant == "full":
                nc.gpsimd.collective_compute(
                    "AllToAll",
                    mybir.AluOpType.bypass,
                    replica_groups=RG,
                    ins=[cc_in[:].opt()],
                    outs=[a2a_out[:].opt()],
                )
            else:
                for s in range(W):
                    nc.sync.dma_start(
                        a2a_out[s * SR : (s + 1) * SR, :], cc_in[s * SR : (s + 1) * SR, :]
                    )

            # ---- local tree-sum of the 8 partials of this core's column-block
            # (two 3D-AP DMAs pull shards 0-3 / 4-7 side by side in the free
            # dim; 3 halving adds reduce them)
            srow = ep.tile([1, N], F32, tag="srow")
            la = alp.tile([RB, 4, N], I16, tag="ld16")
            nc.scalar.dma_start(
                la[:], a2a_out[0 : 4 * SR, :].rearrange("(s p) n -> p s n", p=SR)[0:RB, :, :]
            )
            lb = alp.tile([RB, 4, N], I16, tag="ld16b")
            nc.sync.dma_start(
                lb[:], a2a_out[4 * SR : 8 * SR, :].rearrange("(s p) n -> p s n", p=SR)[0:RB, :, :]
            )
            p4 = ep.tile([RB, 4, N], I16, tag="p4")
            nc.vector.tensor_add(p4[:], la[:], lb[:])
            p2 = ep.tile([RB, 2, N], I16, tag="p2")
            nc.vector.tensor_add(p2[:], p4[:, 0:2, :], p4[:, 2:4, :])
            summ = ep.tile([128, N], F32, tag="summ")
            nc.vector.tensor_add(summ[:], p2[:, 0, :], p2[:, 1, :])
            # area rows: sum the 8 partials with a k=8 matmul (partition
            # contraction), avoiding partition-offset slices
            ar16 = ep.tile([W, N], I16, tag="ar16")
            for s in range(W):
                nc.scalar.dma_start(ar16[s : s + 1, :], a2a_out[s * SR + RB : (s + 1) * SR, :])
            arf = ep.tile([W, N], F32, tag="arf")
            nc.vector.tensor_copy(arf[:], ar16[:])
            ones8 = ep.tile([W, 1], F32, tag="ones8")
            nc.vector.memset(ones8[:], 1.0)
            arp = pgp.tile([1, N], F32, tag="pg", name="arp")
            for off in range(0, N, 512):
                nc.tensor.matmul(
                    arp[:, off : off + 512], ones8[:], arf[:, off : off + 512],
                    start=True, stop=True,
                )
            nc.vector.tensor_copy(srow[:], arp[:])

            # transpose the summed column-block into this core's row stripe
            # (4 blocks per PSUM tile -> 2 big copies instead of 8 small)
            stripe = ep.tile([128, N], F32, tag="stripe")
            for h in range(2):
                tp = pgp.tile([128, 512], F32, tag="pg", name=f"tph{h}")
                for b in range(4):
                    nc.tensor.transpose(
                        tp[:, b * 128 : (b + 1) * 128],
                        summ[:, (4 * h + b) * 128 : (4 * h + b + 1) * 128],
                        ident[:],
                    )
                nc.vector.tensor_copy(stripe[:, h * 512 : (h + 1) * 512], tp[:])

            # ---- epilogue on the stripe
            maskT = ep.tile([128, N], F32, tag="maskT")
            nc.gpsimd.dma_start(maskT[:], maskT_h[:])
            diagsel = ep.tile([128, N], F32, tag="diagsel")
            nc.gpsimd.dma_start(diagsel[:], diagsel_h[:])
            scores = ep.tile([1, RB], F32, tag="scores")
            nc.gpsimd.dma_start(scores[:], scores_h[:])

            # s_col[p] = area of row (128c+p)  (diagonal of the stripe)
            tmp = ep.tile([128, N], F32, tag="e1")
            nc.vector.tensor_mul(tmp[:], stripe[:], diagsel[:])
            s_col = ep.tile([128, 1], F32, tag="s_col")
            nc.vector.tensor_reduce(
                s_col[:], tmp[:], axis=mybir.AxisListType.X, op=mybir.AluOpType.add
            )

            # broadcast s (row) across partitions via k=1 outer matmul
            sj = pgp.tile([128, N], F32, tag="pg")
            for off in range(0, N, 512):
                nc.tensor.matmul(
                    sj[:, off : off + 512], ones_r[:], srow[:, off : off + 512],
                    start=True, stop=True,
                )
            # union = s_i + s_j - inter
            un = ep.tile([128, N], F32, tag="e2")
            nc.vector.tensor_tensor(un[:], sj[:], stripe[:], op=mybir.AluOpType.subtract)
            nc.vector.tensor_scalar(
                un[:], un[:], s_col[:], 1.0,
                op0=mybir.AluOpType.add, op1=mybir.AluOpType.max,
            )
            rec = ep.tile([128, N], F32, tag="e1")
            nc.vector.reciprocal(rec[:], un[:])
            # dmT[p, i] = d[i, 128c+p]  (masked IoU, transposed view via symmetry)
            dmT = ep.tile([128, N], F32, tag="e2")
            nc.vector.tensor_mul(dmT[:], stripe[:], rec[:])
            nc.vector.tensor_mul(dmT[:], dmT[:], maskT[:])
            # compensate_iou for this core's 128 candidates: free-dim max
            c_loc = ep.tile([128, 1], F32, tag="c_loc")
            nc.vector.tensor_reduce(
                c_loc[:], dmT[:], axis=mybir.AxisListType.X, op=mybir.AluOpType.max
            )
            cl_ps = pgp.tile([1, 128], F32, tag="pg", name="cl_ps")
            nc.tensor.transpose(cl_ps[:], c_loc[:], ident[:])
            cl_row = ep.tile([1, 128], F32, tag="cl_row")
            nc.vector.tensor_copy(cl_row[:], cl_ps[:])
            nc.scalar.dma_start(ag_in[:], cl_row[:])
            if variant == "full":
                nc.gpsimd.collective_compute(
                    "AllGather",
                    mybir.AluOpType.bypass,
                    replica_groups=RG,
                    ins=[ag_in[:].opt()],
                    outs=[ag_out[:].opt()],
                )
            else:
                for r in range(W):
                    nc.sync.dma_start(ag_out[r * RB : (r + 1) * RB, :], ag_in[:])
            crow = ep.tile([1, N], F32, tag="crow")
            nc.sync.dma_start(crow[:], ag_out[:])
            c2row = ep.tile([1, N], F32, tag="c2row")
            nc.scalar.square(c2row[:], crow[:])
            c2b = pgp.tile([128, N], F32, tag="pg")
            for off in range(0, N, 512):
                nc.tensor.matmul(
                    c2b[:, off : off + 512], ones_r[:], c2row[:, off : off + 512],
                    start=True, stop=True,
                )
            # f[p, i] = d[i, j]^2 - c[i]^2 ; M_j = max_i f  (j = 128c+p)
            f = ep.tile([128, N], F32, tag="e1")
            nc.scalar.square(f[:], dmT[:])
            nc.vector.tensor_tensor(f[:], f[:], c2b[:], op=mybir.AluOpType.subtract)
            m_loc = ep.tile([128, 1], F32, tag="m_loc")
            nc.vector.tensor_reduce(
                m_loc[:], f[:], axis=mybir.AxisListType.X, op=mybir.AluOpType.max
            )
            # out = scores * exp(-sigma * M), assembled in row space so the
            # store is one contiguous 512B descriptor
            ml_ps = pgp.tile([1, 128], F32, tag="pg", name="ml_ps")
            nc.tensor.transpose(ml_ps[:], m_loc[:], ident[:])
            e_t = ep.tile([1, RB], F32, tag="e_t")
            nc.scalar.activation(
                e_t[:], ml_ps[:], mybir.ActivationFunctionType.Exp, scale=-SIGMA
            )
            outsb = ep.tile([1, RB], F32, tag="outsb")
            nc.vector.tensor_mul(outsb[:], e_t[:], scores[:])
            nc.scalar.dma_start(out_h[:], outsb[:])

    nc.compile()
    return nc


_NC_CACHE = {}


def _get_nc(variant="full"):
    if variant not in _NC_CACHE:
        _NC_CACHE[variant] = build_nc(variant)
    return _NC_CACHE[variant]


def make_in_maps(seg_masks, cate_labels, cate_scores):
    flat = np.ascontiguousarray(np.asarray(seg_masks, dtype=np.float32).reshape(N, -1))
    labels = np.asarray(cate_labels)
    scores = np.asarray(cate_scores, dtype=np.float32)
    xTfull = np.ascontiguousarray(flat.T)  # (40960, 1024)
    gidx = np.arange(N)
    ident = np.eye(128, dtype=np.float32)
    ones_r = np.ones((1, 128), dtype=np.float32)
    in_maps = []
    for c in range(W):
        rows = slice(c * RB, (c + 1) * RB)
        gr = gidx[rows]
        maskT = (
            (gidx[None, :] < gr[:, None]) & (labels[None, :] == labels[rows][:, None])
        ).astype(np.float32)
        diagsel = np.zeros((RB, N), dtype=np.float32)
        diagsel[np.arange(RB), gr] = 1.0
        in_maps.append(
            {
                # partition-major: row p holds k-rows {p, 128+p, ...} of this
                # core's slice, so each DMA descriptor moves 20KB contiguous
                "xT": np.ascontiguousarray(
                    xTfull[c * KC : (c + 1) * KC]
                    .reshape(KT, 128, N)
                    .transpose(1, 0, 2)
                ).reshape(128, KT * N),
                "maskT": maskT,
                "diagsel": diagsel,
                "scores": scores[rows].reshape(1, RB),
                "ident": ident,
                "ones_r": ones_r,
            }
        )
    return in_maps


def run_device(in_maps, trace=False):
    nc = _get_nc()
    res = bass_utils.run_bass_kernel_spmd(
        nc, in_maps, core_ids=list(range(W)), trace=trace
    )
    return res


def kernel(seg_masks, cate_labels, cate_scores):
    in_maps = make_in_maps(seg_masks, cate_labels, cate_scores)
    res = run_device(in_maps)
    outs = [np.asarray(res.results[c]["out"]).reshape(RB) for c in range(W)]
    return np.concatenate(outs).astype(np.float32)


def make_runner(in_maps, nc=None):
    """Device-resident repeated executor (mirrors bass2jax.run_bass_via_pjrt
    multi-core path, but keeps the big inputs on device across calls)."""
    import jax
    from jax.sharding import Mesh, NamedSharding, PartitionSpec
    from jax.experimental.shard_map import shard_map
    from concourse import bass2jax

    if nc is None:
        nc = _get_nc()
    bass2jax.install_neuronx_cc_hook()

    partition_name = nc.partition_id_tensor.name if nc.partition_id_tensor else None
    in_names, out_names, out_avals, zero_outs = [], [], [], []
    for alloc in nc.m.functions[0].allocations:
        if not isinstance(alloc, mybir.MemoryLocationSet):
            continue
        name = alloc.memorylocations[0].name
        if alloc.kind == "ExternalInput":
            if name != partition_name:
                in_names.append(name)
        elif alloc.kind == "ExternalOutput":
            out_names.append(name)
            shape = tuple(alloc.tensor_shape)
            dtype = mybir.dt.np(alloc.dtype)
            out_avals.append(jax.core.ShapedArray(shape, dtype))
            zero_outs.append(np.zeros(shape, dtype))
    n_params = len(in_names)
    n_outs = len(out_avals)
    all_in_names = list(in_names) + list(out_names)
    if partition_name is not None:
        all_in_names.append(partition_name)
    donate = tuple(range(n_params, n_params + n_outs))

    def _body(*args):
        operands = list(args)
        if partition_name is not None:
            operands.append(bass2jax.partition_id_tensor())
        outs = bass2jax._bass_exec_p.bind(
            *operands,
            out_avals=tuple(out_avals),
            in_names=tuple(all_in_names),
            out_names=tuple(out_names),
            lowering_input_output_aliases=(),
            sim_require_finite=True,
            sim_require_nnan=True,
            nc=nc,
        )
        return tuple(outs)

    ncore = len(in_maps)
    devices = jax.devices()[:ncore]
    mesh = Mesh(np.asarray(devices), ("core",))
    in_specs = (PartitionSpec("core"),) * (n_params + n_outs)
    out_specs = (PartitionSpec("core"),) * n_outs
    sharded = jax.jit(
        shard_map(_body, mesh=mesh, in_specs=in_specs, out_specs=out_specs,
                  check_rep=False),
        donate_argnums=donate, keep_unused=True,
    )
    shd = NamedSharding(mesh, PartitionSpec("core"))
    concat_in = [
        np.concatenate([np.asarray(in_maps[c][nm]) for c in range(ncore)], axis=0)
        for nm in in_names
    ]
    dev_in = [jax.device_put(x, shd) for x in concat_in]
    zshapes = [(ncore * z.shape[0], *z.shape[1:]) for z in zero_outs]

    def run():
        dz = [jax.device_put(np.zeros(s, z.dtype), shd)
              for s, z in zip(zshapes, zero_outs)]
        return sharded(*dev_in, *dz)

    def fetch(out_arrs):
        return [
            {nm: np.asarray(out_arrs[i]).reshape(ncore, *out_avals[i].shape)[c]
             for i, nm in enumerate(out_names)}
            for c in range(ncore)
        ]

    return run, fetch


def build_null_nc():
    """Trivial 8-core kernel (one 4KB DMA through SBUF) to measure the
    dispatch + launch floor of the execute path."""
    nc = bacc.Bacc("TRN2", target_bir_lowering=False, debug=False, num_devices=W)
    a_h = nc.dram_tensor("a", [RB, 1], F32, kind="ExternalInput")
    o_h = nc.dram_tensor("out", [RB, 1], F32, kind="ExternalOutput")
    with tile.TileContext(nc) as tc:
        with tc.tile_pool(name="p", bufs=1) as p:
            t = p.tile([RB, 1], F32, tag="t")
            nc.sync.dma_start(t[:], a_h[:])
            nc.sync.dma_start(o_h[:], t[:])
    nc.compile()
    return nc
